# revision 1
# baseline (speedup 1.0000x reference)
"""Trainium2 Bass kernel for nn_DWT_Layer: 3-level 2D db4 DWT (symmetric mode).

Input  x: (16, 3, 1024, 1024) fp32.
Output:   (16, 3, 64, 128, 128) fp32 — the flattened/truncated wavelet pyramid
          [cA3, cH3, cV3, cD3, cH2, cV2, cD2, cH1, cV1, cD1(truncated)].

Sharding: pure data parallel — the 48 (batch*channel) images are split 6 per
NeuronCore across 8 cores; no communication.

Per-core dataflow, per image, per level (N -> N' = floor((N+5)/2)+1):
  1. width pass along the free axis: 8-tap stride-2 MAC chains on the
     vector (DVE) + gpsimd engines over a symmetric-extension buffer
     (ext built by DMA/copies writing the interior + 4 mirror copies).
  2. height pass as banded fp32 matmuls on the tensor engine: stacked
     [lo; hi] folded DWT matrix (symmetric fold absorbed into weights),
     contraction over partitions; only nonzero 128x128 blocks are run.
  3. scalar (ACT) engine copies PSUM -> SBUF, splitting quadrants; the
     aa quadrant lands in the next level's ext buffer, detail quadrants
     land in per-slot staging tiles that stream to DRAM.
Everything comes out h-major so output DMAs are contiguous-row writes.
"""
import numpy as np

# ----------------------------------------------------------------- constants
DEC_LO = np.array([-0.010597401784997278, 0.032883011666982945,
                   0.030841381835986965, -0.18703481171888114,
                   -0.027983769416983849, 0.63088076792959036,
                   0.71484657055254153, 0.23037781330885523], dtype=np.float64)
L = 8
DEC_HI = np.array([(-1.0) ** (k + 1) * DEC_LO[L - 1 - k] for k in range(L)],
                  dtype=np.float64)
FREV_LO = [float(v) for v in DEC_LO[::-1].astype(np.float32)]
FREV_HI = [float(v) for v in DEC_HI[::-1].astype(np.float32)]
TAPS_ARR = np.tile(np.array(FREV_LO + FREV_HI, dtype=np.float32)[None, :],
                   (128, 1))

B, C, H, W = 16, 3, 1024, 1024
N_CORES = 8
IMGS_PER_CORE = 6
IMG_ELEMS = H * W

LEVELS = [  # (N, N', n_slots_in, n_out_tiles)
    (1024, 515, 8, 9),
    (515, 261, 5, 5),
    (261, 134, 3, 3),
]

# output section offsets (elements within one image's 1048576-long output)
SECT = {}
_cur = 0
for _name, _n in [("cA3", 134), ("cH3", 134), ("cV3", 134), ("cD3", 134),
                  ("cH2", 261), ("cV2", 261), ("cD2", 261),
                  ("cH1", 515), ("cV1", 515), ("cD1", 515)]:
    SECT[_name] = (_cur, _n)
    _cur += _n * _n
# cD1 truncation: keep first 469 full rows + 404 elems of row 469
CD1_FULL_ROWS = 469
CD1_PART_COLS = 404
assert SECT["cD1"][0] + CD1_FULL_ROWS * 515 + CD1_PART_COLS == IMG_ELEMS


def nprime(N):
    return (N + 5) // 2 + 1


def ext_index(j, N):
    if j < 6:
        return 5 - j
    if j < N + 6:
        return j - 6
    return 2 * N + 5 - j


def dwt_matrix(N, filt):
    Np = nprime(N)
    M = np.zeros((Np, N), dtype=np.float64)
    filtrev = filt[::-1]
    for i in range(Np):
        for t in range(L):
            M[i, ext_index(2 * i + t, N)] += filtrev[t]
    return M


def hi_off(Np):
    """row offset of the hi section, padded to a multiple of 32 so that
    engine ops on the hi quadrant start at partition 32/64/0 (BIR verifier:
    SBUF engine APs must start at partition 0/32/64/96)."""
    return ((Np + 31) // 32) * 32


def stacked_matrix(N):
    Np = nprime(N)
    off = hi_off(Np)
    M2 = np.zeros((off + Np, N), dtype=np.float64)
    M2[0:Np] = dwt_matrix(N, DEC_LO)
    M2[off:] = dwt_matrix(N, DEC_HI)
    return M2.astype(np.float32)


def band_blocks(N):
    """[(t, q, kq, mt, band_pos)]: nonzero blocks of M2^T; band_pos tags
    first/last per (t) for start/stop flags."""
    M2 = stacked_matrix(N)
    R = M2.shape[0]
    kt = (N + 127) // 128
    ot = (R + 127) // 128
    per_t = []
    for t in range(ot):
        qs = []
        for q in range(kt):
            blk = M2[t * 128:(t + 1) * 128, q * 128:(q + 1) * 128]
            if np.any(blk != 0):
                qs.append(q)
        per_t.append(qs)
    return per_t, kt, ot, R


def const_weights(N):
    """packed lhsT blocks [128, nblocks, 128] + index map {(t,q): b}."""
    M2 = stacked_matrix(N)
    per_t, kt, ot, R = band_blocks(N)
    blocks = [(t, q) for t in range(ot) for q in per_t[t]]
    arr = np.zeros((128, len(blocks), 128), dtype=np.float32)
    idx = {}
    for b, (t, q) in enumerate(blocks):
        blk = M2[t * 128:(t + 1) * 128, q * 128:(q + 1) * 128]  # [mt, kq]
        arr[:blk.shape[1], b, :blk.shape[0]] = blk.T
        idx[(t, q)] = b
    return arr, idx, per_t


WC = {N: const_weights(N) for N, _, _, _ in LEVELS}

# ---- MAC pass tuning knobs ----
GP_FRAC = 0.0     # fraction of width-pass columns offloaded to gpsimd (Pool)
MAC_SPLIT = 1     # independent DVE chains per filter (hides RAW ack latency)
TAP0_ACT = True   # first tap (overwrite mul) on the scalar engine
MIRROR_GP = True  # mirror/memset ops on gpsimd instead of DVE
TAP_MAJOR = True  # emit MAC ops tap-major (interleave chains) vs unit-major
EXT1_BUFS = 3
WB1_BUFS = 3
EXT2_BUFS = 1
WB2_BUFS = 1
DET_BUFS = 6

_BUILT = None  # cached (nc, meta)


def _free_chunks(Np):
    """quadrant-aligned free chunks of <=512: [(c0, c1), ...] covering
    [0:2Np). Each chunk lies in one quadrant and fits one PSUM bank."""
    out = []
    for base in (0, Np):
        c = 0
        while c < Np:
            e = min(c + 512, Np)
            out.append((base + c, base + e))
            c = e
    return out


def _emit_mirror_ops(nc, ext, S, N):
    """Fill ext cols [0:6) and [N+6:N+13) from the interior [6:N+6)."""
    eng = nc.gpsimd if MIRROR_GP else nc.vector
    # left: ext[j] = x[5-j] = ext[6 + 5 - j] -> reversed slice of cols [6:12)
    eng.tensor_copy(out=ext[:, 0:S, 0:6], in_=ext[:, 0:S, 11:5:-1])
    # right: ext[N+6+k] = x[N-1-k] at ext col N+5-k -> reversed (N+5 .. N-1)
    eng.tensor_copy(out=ext[:, 0:S, N + 6:N + 13],
                    in_=ext[:, 0:S, N + 5:N - 2:-1])


def _emit_mac_pass(nc, ext, wb, S, N, Np, taps_sb, tmp_pool, lvl):
    """width pass: wb[:, s, c + base] = sum_t frev[t] * ext[:, s, 2c+t].

    DVE runs fused multiply-accumulate (scalar_tensor_tensor) chains;
    a GP_FRAC column share goes to gpsimd as mult+add pairs (walrus
    rejects TensorScalarPtr on Pool). Ops are emitted tap-major so
    independent chains interleave and hide the RAW pipeline latency."""
    import concourse.mybir as mybir
    gp_n = int(Np * GP_FRAC)
    dve_n = Np - gp_n
    units = []  # (kind, fi, c0, c1, tmp)
    nsub = max(1, MAC_SPLIT)
    bounds = [round(dve_n * i / nsub) for i in range(nsub + 1)]
    for fi in (0, 1):
        for si in range(nsub):
            if bounds[si] < bounds[si + 1]:
                units.append(("v", fi, bounds[si], bounds[si + 1], None))
        if gp_n > 0:
            tmp = tmp_pool.tile([128, S, gp_n], mybir.dt.float32,
                                tag=f"gtmp{lvl}", bufs=3,
                                name=f"gtmp{lvl}_{fi}")
            units.append(("g", fi, dve_n, Np, tmp))

    order = ([(t, u) for t in range(L) for u in units] if TAP_MAJOR
             else [(t, u) for u in units for t in range(L)])
    for t, u in order:
        if True:
            kind, fi, c0, c1, tmp = u
            frev = FREV_LO if fi == 0 else FREV_HI
            n = c1 - c0
            base = fi * Np
            src = ext[:, 0:S, 2 * c0 + t: 2 * c0 + t + 2 * (n - 1) + 1: 2]
            dst = wb[:, 0:S, base + c0: base + c1]
            if kind == "v":
                if t == 0:
                    if TAP0_ACT:
                        nc.scalar.mul(dst, src, frev[t])
                    else:
                        nc.vector.tensor_scalar_mul(dst, src, frev[t])
                else:
                    nc.vector.scalar_tensor_tensor(
                        out=dst, in0=src, scalar=frev[t], in1=dst,
                        op0=mybir.AluOpType.mult, op1=mybir.AluOpType.add)
            else:
                btap = taps_sb[:, fi * 8 + t:fi * 8 + t + 1].to_broadcast(
                    (128, S, n))
                if t == 0:
                    nc.gpsimd.tensor_tensor(out=dst, in0=src, in1=btap,
                                            op=mybir.AluOpType.mult)
                else:
                    nc.gpsimd.tensor_tensor(out=tmp[:, 0:S, 0:n], in0=src,
                                            in1=btap, op=mybir.AluOpType.mult)
                    nc.gpsimd.tensor_tensor(out=dst, in0=dst,
                                            in1=tmp[:, 0:S, 0:n],
                                            op=mybir.AluOpType.add)


def build_bass(n_images=IMGS_PER_CORE, repeats=1):
    import concourse.mybir as mybir
    import concourse.tile as tile
    from concourse import bacc
    from contextlib import ExitStack

    nc = bacc.Bacc("TRN2", target_bir_lowering=False, debug=False)

    xin = nc.dram_tensor("xin", (n_images, H, W), mybir.dt.float32,
                         kind="ExternalInput").ap()
    out = nc.dram_tensor("out", (n_images, IMG_ELEMS), mybir.dt.float32,
                         kind="ExternalOutput").ap()
    wdram = {}
    for N, _, _, _ in LEVELS:
        arr, _, _ = WC[N]
        wdram[N] = nc.dram_tensor(f"w{N}", arr.shape, mybir.dt.float32,
                                  kind="ExternalInput").ap()
    taps_dram = nc.dram_tensor("taps", (128, 16), mybir.dt.float32,
                               kind="ExternalInput").ap()

    with tile.TileContext(nc) as tc, ExitStack() as ctx:
        cpool = ctx.enter_context(tc.tile_pool(name="consts", bufs=1))
        extp = ctx.enter_context(tc.tile_pool(name="ext", bufs=1))
        wbp = ctx.enter_context(tc.tile_pool(name="wb", bufs=1))
        psp = ctx.enter_context(tc.tile_pool(name="ps", bufs=1, space="PSUM"))
        detp = ctx.enter_context(tc.tile_pool(name="det", bufs=1))

        wsb = {}
        for N, _, _, _ in LEVELS:
            arr, _, _ = WC[N]
            wsb[N] = cpool.tile(list(arr.shape), mybir.dt.float32,
                                name=f"wsb{N}")
            nc.sync.dma_start(out=wsb[N][:], in_=wdram[N])
        taps_sb = cpool.tile([128, 16], mybir.dt.float32, name="taps_sb")
        nc.sync.dma_start(out=taps_sb[:], in_=taps_dram)

        for _rep in range(repeats):
            for img in range(n_images):
                _emit_image(nc, tc, extp, wbp, psp, detp, wsb, taps_sb,
                            xin, out, img)

    nc.compile()
    return nc


def _emit_image(nc, tc, extp, wbp, psp, detp, wsb, taps_sb, xin, out, img):
    import concourse.mybir as mybir

    N1, P1 = 1024, 515
    # ---------------- L1: ext halves + MACs ----------------
    halves = []
    for h in range(2):
        ext = extp.tile([128, 4, N1 + 13], mybir.dt.float32, tag="ext1",
                        bufs=EXT1_BUFS, name=f"ext1_{img}_{h}")
        src = xin[img, 512 * h:512 * (h + 1), :].rearrange(
            "(s p) w -> p s w", p=128)
        nc.sync.dma_start(out=ext[:, 0:4, 6:N1 + 6], in_=src)
        _emit_mirror_ops(nc, ext, 4, N1)
        wb = wbp.tile([128, 4, 2 * P1], mybir.dt.float32, tag="wb1",
                      bufs=WB1_BUFS, name=f"wb1_{img}_{h}")
        _emit_mac_pass(nc, ext, wb, 4, N1, P1, taps_sb, wbp, 1)
        halves.append(wb)

    def rhs1(q, c0, c1):
        return halves[q // 4][:, q % 4, c0:c1]

    # next-level ext buffers; memset the partial last slot so the unwritten
    # partitions (beyond the valid rows) are finite zeros
    ext2 = extp.tile([128, 5, 515 + 13], mybir.dt.float32, tag="ext2",
                     bufs=EXT2_BUFS, name=f"ext2_{img}")
    (nc.gpsimd if MIRROR_GP else nc.vector).memset(ext2[:, 4, :], 0.0)
    ext3 = extp.tile([128, 3, 261 + 13], mybir.dt.float32, tag="ext3",
                     bufs=EXT2_BUFS, name=f"ext3_{img}")
    (nc.gpsimd if MIRROR_GP else nc.vector).memset(ext3[:, 2, :], 0.0)

    _emit_level_mm(nc, psp, detp, wsb, out, img, N=1024, rhs=rhs1,
                   next_ext=ext2, det_names=("cH1", "cV1", "cD1"))
    _emit_mirror_ops(nc, ext2, 5, 515)

    wb2 = wbp.tile([128, 5, 2 * 261], mybir.dt.float32, tag="wb2",
                   bufs=WB2_BUFS, name=f"wb2_{img}")
    _emit_mac_pass(nc, ext2, wb2, 5, 515, 261, taps_sb, wbp, 2)

    def rhs2(q, c0, c1):
        return wb2[:, q, c0:c1]

    _emit_level_mm(nc, psp, detp, wsb, out, img, N=515, rhs=rhs2,
                   next_ext=ext3, det_names=("cH2", "cV2", "cD2"))
    _emit_mirror_ops(nc, ext3, 3, 261)

    wb3 = wbp.tile([128, 3, 2 * 134], mybir.dt.float32, tag="wb3",
                   bufs=WB2_BUFS, name=f"wb3_{img}")
    _emit_mac_pass(nc, ext3, wb3, 3, 261, 134, taps_sb, wbp, 3)

    def rhs3(q, c0, c1):
        return wb3[:, q, c0:c1]

    _emit_level_mm(nc, psp, detp, wsb, out, img, N=261, rhs=rhs3,
                   next_ext=None, det_names=("cH3", "cV3", "cD3"))


def _emit_level_mm(nc, psp, detp, wsb, out, img, N, rhs, next_ext, det_names):
    """height-pass matmuls + psum->sbuf quadrant copies + detail DMAs."""
    import concourse.mybir as mybir

    Np = nprime(N)
    arr, idx, per_t = WC[N]
    OFF = hi_off(Np)
    R = OFF + Np
    ot = (R + 127) // 128
    kN = N  # contraction length
    chunks = _free_chunks(Np)

    for t in range(ot):
        mt = min(128, R - t * 128)
        qs = per_t[t]
        ps_tiles = []
        for ci, (c0, c1) in enumerate(chunks):
            w = c1 - c0
            tag = "psA" if w > 256 else "psB"
            ps = psp.tile([128, w], mybir.dt.float32, tag=tag, bufs=4,
                          name=f"ps_{img}_{N}_{t}_{ci}")
            ps_tiles.append(ps)
            for ki, q in enumerate(qs):
                kq = min(128, kN - q * 128)
                r = rhs(q, c0, c1)
                if kq < 128:
                    r = r[0:kq]
                nc.tensor.matmul(
                    ps[0:mt, 0:w],
                    wsb[N][0:kq, idx[(t, q)], 0:mt],
                    r,
                    start=(ki == 0), stop=(ki == len(qs) - 1))

        # quadrant qd -> list of (ps_tile, dst_col0, width)
        quad_srcs = {0: [], 1: []}
        for ci, (c0, c1) in enumerate(chunks):
            qd = 0 if c0 < Np else 1
            quad_srcs[qd].append((ps_tiles[ci], c0 - qd * Np, c1 - c0))

        # lo rows: global [0:Np); hi rows: global [OFF:OFF+Np)
        lo_end = min(128, Np - t * 128) if t * 128 < Np else 0
        hp0 = max(0, OFF - t * 128)
        hp1 = max(0, min(128, OFF + Np - t * 128))
        # split hi ranges at legal partition starts (0/32/64)
        hi_ranges = []
        if hp0 < hp1:
            if hp0 == 0:
                hi_ranges = [(0, hp1)]
            else:
                assert hp0 == 32, hp0
                hi_ranges = [(32, min(64, hp1))]
                if hp1 > 64:
                    hi_ranges.append((64, hp1))

        if lo_end > 0:
            # quadrant 0 = aa -> next level ext (or cA3 staging tile)
            if next_ext is not None:
                for ps, d0, w in quad_srcs[0]:
                    nc.scalar.copy(out=next_ext[0:lo_end, t, 6 + d0:6 + d0 + w],
                                   in_=ps[0:lo_end, 0:w])
            else:
                _emit_det_copy_dma(nc, detp, out, img, "cA3", Np,
                                   quad_srcs[0], t, [(0, lo_end)], 0)
            # quadrant 1 = ad = cV
            _emit_det_copy_dma(nc, detp, out, img, det_names[1], Np,
                               quad_srcs[1], t, [(0, lo_end)], 0)
        if hi_ranges:
            # hi rows: da = cH (quadrant 0), dd = cD (quadrant 1)
            _emit_det_copy_dma(nc, detp, out, img, det_names[0], Np,
                               quad_srcs[0], t, hi_ranges, OFF)
            _emit_det_copy_dma(nc, detp, out, img, det_names[2], Np,
                               quad_srcs[1], t, hi_ranges, OFF)


def _emit_det_copy_dma(nc, detp, out, img, sec_name, Np, srcs, t, pranges,
                       row_off):
    """Copy psum chunks into a staging tile, then DMA rows to DRAM.

    h (row index within the detail) = 128*t + p - row_off for partition p.
    pranges: list of legal-start partition ranges covering this tile's rows."""
    import concourse.mybir as mybir
    sec_base, Wd = SECT[sec_name]
    assert Wd == Np
    p0, p1 = pranges[0][0], pranges[-1][1]
    h0 = 128 * t + p0 - row_off
    h1 = h0 + (p1 - p0)
    assert 0 <= h0 and h1 <= Np, (sec_name, t, pranges, h0, h1)

    is_cd1 = sec_name == "cD1"
    if is_cd1 and h0 >= CD1_FULL_ROWS + 1:
        return  # fully truncated
    dt = detp.tile([128, Np], mybir.dt.float32, tag=f"det{Np}", bufs=DET_BUFS,
                   name=f"det_{sec_name}_{img}_{t}_{p0}")
    for ps, d0, w in srcs:
        for (a, b) in pranges:
            nc.scalar.copy(out=dt[a:b, d0:d0 + w], in_=ps[a:b, 0:w])

    full_h1 = h1
    if is_cd1 and h1 > CD1_FULL_ROWS:
        full_h1 = CD1_FULL_ROWS
    if full_h1 > h0:
        npart = full_h1 - h0
        dst = out[img, sec_base + h0 * Wd: sec_base + full_h1 * Wd].rearrange(
            "(h w) -> h w", w=Wd)
        nc.sync.dma_start(out=dst, in_=dt[p0:p0 + npart, :])
    if is_cd1 and h0 <= CD1_FULL_ROWS < h1:
        pp = p0 + (CD1_FULL_ROWS - h0)
        dst = out[img, sec_base + CD1_FULL_ROWS * Wd:
                  sec_base + CD1_FULL_ROWS * Wd + CD1_PART_COLS]
        nc.sync.dma_start(out=dst.rearrange("(h w) -> h w", w=CD1_PART_COLS),
                          in_=dt[pp:pp + 1, 0:CD1_PART_COLS])


# ----------------------------------------------------------------- runner
def _get_built():
    global _BUILT
    if _BUILT is None:
        _BUILT = build_bass()
    return _BUILT


def kernel(x: np.ndarray) -> np.ndarray:
    from concourse import bass_utils

    x = np.ascontiguousarray(np.asarray(x), dtype=np.float32)
    assert x.shape == (B, C, H, W), x.shape
    nc = _get_built()

    imgs = x.reshape(B * C, H, W)
    in_maps = []
    for c in range(N_CORES):
        m = {"xin": imgs[c * IMGS_PER_CORE:(c + 1) * IMGS_PER_CORE]}
        for N, _, _, _ in LEVELS:
            m[f"w{N}"] = WC[N][0]
        m["taps"] = TAPS_ARR
        in_maps.append(m)

    res = bass_utils.run_bass_kernel_spmd(nc, in_maps,
                                          core_ids=list(range(N_CORES)))
    outs = [res.results[c]["out"] for c in range(N_CORES)]
    flat = np.concatenate(outs, axis=0)  # [48, 1048576]
    return flat.reshape(B, C, 64, 128, 128)



# revision 36
# speedup vs baseline: 4.3282x; 4.3282x over previous
"""Trainium2 Bass kernel for nn_DWT_Layer: 3-level 2D db4 DWT (symmetric mode).

Input  x: (16, 3, 1024, 1024) fp32.
Output:   (16, 3, 64, 128, 128) fp32 — the flattened/truncated wavelet pyramid
          [cA3, cH3, cV3, cD3, cH2, cV2, cD2, cH1, cV1, cD1(truncated)].

Sharding: pure data parallel — the 48 (batch*channel) images are split 6 per
NeuronCore across 8 cores; no communication.

All compute runs on the tensor engine in fp16 (1 PE cycle per output row at
any free size). The separable transform per level is two banded matmul
passes with the symmetric extension folded into the weights:

  pass 1 (H):  A^T = X^T · M^T   — lhsT = a 128-col block of X (stationary),
               rhs = a [128, <=64] block of the folded DWT matrix M^T.
               Swapping the stationary operand makes the output land
               TRANSPOSED (image columns on psum partitions), which is
               exactly what pass 2 needs.
  pass 2 (W):  out = A · M^T     — lhsT = a 128-row block of A^T, rhs = the
               SAME weight blocks; output is row-major [h', w'], so the four
               quadrants stream straight to per-section staging tiles and
               then to DRAM in a handful of large DMAs.

Free-dim chunks are 64 wide so each chunk's 8-tap band touches at most two
128-row contraction blocks (2 PE cycles per output element). Intermediates,
weights and output staging are fp16 (validated ~7e-4 rel err vs the fp32
reference); psum accumulation is fp32.
"""
import numpy as np

# ----------------------------------------------------------------- constants
DEC_LO = np.array([-0.010597401784997278, 0.032883011666982945,
                   0.030841381835986965, -0.18703481171888114,
                   -0.027983769416983849, 0.63088076792959036,
                   0.71484657055254153, 0.23037781330885523], dtype=np.float64)
L = 8
DEC_HI = np.array([(-1.0) ** (k + 1) * DEC_LO[L - 1 - k] for k in range(L)],
                  dtype=np.float64)

B, C, H, W = 16, 3, 1024, 1024
N_CORES = 8
IMGS_PER_CORE = 6
IMG_ELEMS = H * W
CH = 32          # free-dim chunk width for the banded matmuls

LEVELS = [  # (N, N', n_slots_in, n_out_tiles) — first field used; rest compat
    (1024, 515, 8, 9),
    (515, 261, 5, 5),
    (261, 134, 3, 3),
]

# output section offsets (elements within one image's 1048576-long output)
SECT = {}
_cur = 0
for _name, _n in [("cA3", 134), ("cH3", 134), ("cV3", 134), ("cD3", 134),
                  ("cH2", 261), ("cV2", 261), ("cD2", 261),
                  ("cH1", 515), ("cV1", 515), ("cD1", 515)]:
    SECT[_name] = (_cur, _n)
    _cur += _n * _n
# cD1 truncation: keep first 469 full rows + 404 elems of row 469
CD1_FULL_ROWS = 469
CD1_PART_COLS = 404
assert SECT["cD1"][0] + CD1_FULL_ROWS * 515 + CD1_PART_COLS == IMG_ELEMS


def nprime(N):
    return (N + 5) // 2 + 1


def ext_index(j, N):
    if j < 6:
        return 5 - j
    if j < N + 6:
        return j - 6
    return 2 * N + 5 - j


def dwt_matrix(N, filt):
    Np = nprime(N)
    M = np.zeros((Np, N), dtype=np.float64)
    filtrev = filt[::-1]
    for i in range(Np):
        for t in range(L):
            M[i, ext_index(2 * i + t, N)] += filtrev[t]
    return M


def level_plan(N):
    """Chunk/weight-block tables for one level (identical for H and W axes).

    chunks: [(sec, out0, w, [(q, bidx)])] over sec in {0:lo, 1:hi}, 64-wide
    output chunks; warr [128, nblk, CH] packs rhs blocks (contraction rows on
    partitions). groups: chunk-aligned psum column groups <= 512 wide over
    the stacked free axis (lo at [0,Np), hi at [Np,2Np))."""
    Np = nprime(N)
    n_cb = (N + 127) // 128
    mats = (dwt_matrix(N, DEC_LO), dwt_matrix(N, DEC_HI))
    chunks = []
    blocks = []
    for sec in (0, 1):
        M = mats[sec]
        for out0 in range(0, Np, CH):
            w = min(CH, Np - out0)
            qs = []
            for q in range(n_cb):
                qn = min(128, N - 128 * q)
                blk = M[out0:out0 + w, 128 * q:128 * q + qn]
                if np.any(blk != 0):
                    qs.append((q, len(blocks)))
                    blocks.append((qn, w, blk.T.copy()))
            chunks.append((sec, out0, w, qs))
    warr = np.zeros((128, len(blocks), CH), dtype=np.float16)
    for b, (qn, w, data) in enumerate(blocks):
        warr[:qn, b, :w] = data.astype(np.float16)
    # psum groups: chunk-aligned, <=512 wide (each fits one psum bank)
    groups = []
    cur0, cur = None, 0
    for (sec, out0, w, qs) in chunks:
        col0 = sec * Np + out0
        if cur0 is None:
            cur0, cur = col0, w
        elif col0 == cur0 + cur and cur + w <= 512:
            cur += w
        else:
            groups.append((cur0, cur))
            cur0, cur = col0, w
    groups.append((cur0, cur))
    return dict(N=N, Np=Np, n_cb=n_cb, chunks=chunks, warr=warr, groups=groups)


PLANS = {N: level_plan(N) for N, _, _, _ in LEVELS}
WC = {N: (PLANS[N]["warr"],) for N, _, _, _ in LEVELS}   # test.py compat
TAPS_ARR = np.zeros((128, 16), dtype=np.float32)          # unused; compat

_BUILT = None


class _CopySched:
    """Greedy least-loaded assignment of psum->sbuf copies across engines."""

    def __init__(self, nc):
        self.nc = nc
        self.busy = {"dve": 0.0, "act": 0.0}
        self.cost = {
            "dve": lambda e: e * 1.0417 + 130.0,
            "act": lambda e: e * 0.8333 + 190.0,
        }

    MODE = "alt"   # "greedy" | "alt"

    def copy(self, out_ap, in_ap, els):
        if self.MODE == "alt":
            eng = "dve" if self.busy["dve"] <= self.busy["act"] else "act"
        else:
            eng = min(self.busy, key=lambda k: self.busy[k] + self.cost[k](els))
        self.busy[eng] += self.cost[eng](els)
        if eng == "dve":
            self.nc.vector.tensor_copy(out=out_ap, in_=in_ap)
        else:
            self.nc.scalar.copy(out=out_ap, in_=in_ap)


def _emit_level(nc, sched, psp, wsb, plan, Xsb, At, quad_dst, img, lvl):
    """One DWT level: pass-1 (H) into At, pass-2 (W) into quadrant tiles.

    Xsb: input tile [128, n_cb, >=N] fp16, rows r = 128*s + p, cols [0,N).
    At:  [128, n_cb, 2*Np] fp16, At[p, cb, h'] = A[h', 128*cb + p].
    quad_dst[sh] = combined tile [128, n_pb, 2*Np], rows 128*pbr + p, cols
    stacked [lo-w | hi-w] matching the psum column layout.
    """
    N, Np, n_cb = plan["N"], plan["Np"], plan["n_cb"]
    chunks, groups = plan["chunks"], plan["groups"]

    _ps_ctr = [0]

    def group_tiles(kind):
        out = []
        for gi, (g0, gw) in enumerate(groups):
            import concourse.mybir as mybir
            width = 512 if gw > 64 else 64
            t = psp.tile([128, width], mybir.dt.float32, tag=f"psg{gi}",
                         bufs=3 if gi < 2 else 2,
                         name=f"ps{kind}{lvl}_{img}_{gi}_{_ps_ctr[0]}")
            _ps_ctr[0] += 1
            out.append((g0, gw, t))
        return out

    def run_chunks(ps_tiles, lhsT_of, mt):
        """Emit the banded matmuls for every chunk into the group tiles."""
        for (g0, gw, ps) in ps_tiles:
            for (sec, out0, w, qs) in chunks:
                col0 = sec * Np + out0
                if not (g0 <= col0 < g0 + gw):
                    continue
                for ki, (q, bidx) in enumerate(qs):
                    qn = min(128, N - 128 * q)
                    nc.tensor.matmul(
                        ps[0:mt, col0 - g0:col0 - g0 + w],
                        lhsT_of(q, qn),
                        wsb[0:qn, bidx, 0:w],
                        start=(ki == 0), stop=(ki == len(qs) - 1))

    # ---------------- pass 1: A^T[c, h'] ----------------
    for cb in range(n_cb):
        cw = min(128, N - 128 * cb)
        ps_tiles = group_tiles("1")
        run_chunks(ps_tiles,
                   lambda q, qn: Xsb[0:qn, q, 128 * cb:128 * cb + cw], cw)
        for (g0, gw, ps) in ps_tiles:
            sched.copy(At[0:cw, cb, g0:g0 + gw], ps[0:cw, 0:gw], gw)

    # ---------------- pass 2: out[h', w'] ----------------
    n_pb = (Np + 127) // 128
    for sh in (0, 1):
        dst = quad_dst[sh]   # combined [128, n_pb, 2*Np]: lo-w | hi-w halves
        for pbr in range(n_pb):
            a = sh * Np + 128 * pbr
            pw = min(128, Np - 128 * pbr)
            ps_tiles = group_tiles("2")
            run_chunks(ps_tiles,
                       lambda q, qn: At[0:qn, q, a:a + pw], pw)
            for (g0, gw, ps) in ps_tiles:
                sched.copy(dst[0:pw, pbr, g0:g0 + gw], ps[0:pw, 0:gw], gw)


def _emit_section_dmas(nc, out, img, name, stg, c0, overshoot=False):
    """DMA one output section from staging cols [c0, c0+Wd) of `stg`.

    overshoot=True rounds the row count up to a slot multiple in ONE DMA;
    the spill rows land in the next DRAM section, whose own (later-emitted)
    DMA overwrites them. Only valid when that section's DMA is emitted
    after this one."""
    base, Wd = SECT[name]
    sl = stg[:, :, c0:c0 + Wd]
    if name == "cD1":
        # rows 0..383 bulk, slot-3 rows 384..468, partial row 469 (404 cols)
        dst = out[img, base:base + 3 * 128 * Wd].rearrange(
            "(s p w) -> p s w", p=128, s=3)
        nc.sync.dma_start(out=dst, in_=sl[:, 0:3, :])
        n85 = CD1_FULL_ROWS - 384
        dst = out[img, base + 384 * Wd:base + CD1_FULL_ROWS * Wd].rearrange(
            "(p w) -> p w", w=Wd)
        nc.sync.dma_start(out=dst, in_=sl[0:n85, 3, :])
        dst = out[img, base + CD1_FULL_ROWS * Wd:
                  base + CD1_FULL_ROWS * Wd + CD1_PART_COLS]
        nc.sync.dma_start(out=dst.rearrange("(p w) -> p w", w=CD1_PART_COLS),
                          in_=sl[n85:n85 + 1, 3, 0:CD1_PART_COLS])
        return
    fs, rem = Wd // 128, Wd % 128
    if rem and overshoot:
        dst = out[img, base:base + (fs + 1) * 128 * Wd].rearrange(
            "(s p w) -> p s w", p=128, s=fs + 1)
        nc.sync.dma_start(out=dst, in_=sl[:, 0:fs + 1, :])
        return
    dst = out[img, base:base + fs * 128 * Wd].rearrange(
        "(s p w) -> p s w", p=128, s=fs)
    nc.sync.dma_start(out=dst, in_=sl[:, 0:fs, :])
    if rem:
        dst = out[img, base + fs * 128 * Wd:base + Wd * Wd].rearrange(
            "(p w) -> p w", w=Wd)
        nc.sync.dma_start(out=dst, in_=sl[0:rem, fs, :])


def build_bass(n_images=IMGS_PER_CORE, repeats=1):
    import concourse.mybir as mybir
    import concourse.tile as tile
    from concourse import bacc
    from contextlib import ExitStack

    nc = bacc.Bacc("TRN2", target_bir_lowering=False, debug=False)
    f16 = mybir.dt.float16

    xin = nc.dram_tensor("xin", (n_images, H, W), f16,
                         kind="ExternalInput").ap()
    out = nc.dram_tensor("out", (n_images, IMG_ELEMS), f16,
                         kind="ExternalOutput").ap()
    wdram = {}
    for N, _, _, _ in LEVELS:
        arr = PLANS[N]["warr"]
        wdram[N] = nc.dram_tensor(f"w{N}", arr.shape, f16,
                                  kind="ExternalInput").ap()

    with tile.TileContext(nc) as tc, ExitStack() as ctx:
        cpool = ctx.enter_context(tc.tile_pool(name="consts", bufs=1))
        xp = ctx.enter_context(tc.tile_pool(name="xp", bufs=1))
        atp = ctx.enter_context(tc.tile_pool(name="atp", bufs=1))
        stp = ctx.enter_context(tc.tile_pool(name="stp", bufs=1))
        psp = ctx.enter_context(tc.tile_pool(name="ps", bufs=1, space="PSUM"))

        x1_tiles = {}

        def get_x1(rep, img):
            if img >= n_images:
                return None
            if (rep, img) not in x1_tiles:
                t = xp.tile([128, 8, 1024], f16, tag="x1", bufs=3,
                            name=f"x1_{rep}_{img}")
                src = xin[img].rearrange("(s p) w -> p s w", p=128)
                # two halves so the first pass-1 chunks can start early
                nc.sync.dma_start(out=t[:, 0:4, :], in_=src[:, 0:4, :])
                nc.sync.dma_start(out=t[:, 4:8, :], in_=src[:, 4:8, :])
                x1_tiles[(rep, img)] = t
            return x1_tiles[(rep, img)]

        wsb = {}

        def load_w(N):
            arr = PLANS[N]["warr"]
            wsb[N] = cpool.tile(list(arr.shape), f16, name=f"wsb{N}")
            # contiguous per-partition transfer (4KB runs, not 128B)
            nc.sync.dma_start(
                out=wsb[N][:].rearrange("p b c -> p (b c)"),
                in_=wdram[N].rearrange("p b c -> p (b c)"))

        load_w(1024)   # needed first
        get_x1(0, 0)   # first image input right behind it
        load_w(515)
        load_w(261)

        seq = [(r, i) for r in range(repeats) for i in range(n_images)]
        sched = _CopySched(nc)
        lo1s, lo2s = {}, {}

        def emit_L1(k):
            rep, img = seq[k]
            x1 = get_x1(rep, img)
            # combined lo/hi pair tiles: lo = [aa | cV], hi = [cH | cD];
            # the aa half (cols [0,Np)) doubles as the next level's input.
            at1 = atp.tile([128, 8, 1030], f16, tag="at1", bufs=2,
                           name=f"at1_{rep}_{img}")
            lo1 = xp.tile([128, 5, 1030], f16, tag="lo1", bufs=3,
                          name=f"lo1_{rep}_{img}")
            hi1 = stp.tile([128, 5, 1030], f16, tag="hi1", bufs=2,
                           name=f"hi1_{rep}_{img}")
            _emit_level(nc, sched, psp, wsb[1024], PLANS[1024], x1, at1,
                        {0: lo1, 1: hi1}, img, 1)
            if k + 1 < len(seq):   # prefetch next image's input
                get_x1(*seq[k + 1])
            osh = k + 1 < len(seq)
            _emit_section_dmas(nc, out, img, "cH1", hi1, 0, overshoot=osh)
            _emit_section_dmas(nc, out, img, "cV1", lo1, 515, overshoot=osh)
            _emit_section_dmas(nc, out, img, "cD1", hi1, 515)
            lo1s[k] = lo1

        def emit_L2(k):
            rep, img = seq[k]
            osh = k + 1 < len(seq)
            at2 = atp.tile([128, 5, 522], f16, tag="at2", bufs=2,
                           name=f"at2_{rep}_{img}")
            lo2 = xp.tile([128, 3, 522], f16, tag="lo2", bufs=3,
                          name=f"lo2_{rep}_{img}")
            hi2 = stp.tile([128, 3, 522], f16, tag="hi2", bufs=2,
                           name=f"hi2_{rep}_{img}")
            _emit_level(nc, sched, psp, wsb[515], PLANS[515], lo1s.pop(k),
                        at2, {0: lo2, 1: hi2}, img, 2)
            _emit_section_dmas(nc, out, img, "cH2", hi2, 0, overshoot=osh)
            _emit_section_dmas(nc, out, img, "cV2", lo2, 261, overshoot=osh)
            _emit_section_dmas(nc, out, img, "cD2", hi2, 261)
            lo2s[k] = lo2

        def emit_L3(k):
            rep, img = seq[k]
            osh = k + 1 < len(seq)
            at3 = atp.tile([128, 3, 268], f16, tag="at3", bufs=2,
                           name=f"at3_{rep}_{img}")
            lo3 = stp.tile([128, 2, 268], f16, tag="lo3", bufs=2,
                           name=f"lo3_{rep}_{img}")
            hi3 = stp.tile([128, 2, 268], f16, tag="hi3", bufs=2,
                           name=f"hi3_{rep}_{img}")
            _emit_level(nc, sched, psp, wsb[261], PLANS[261], lo2s.pop(k),
                        at3, {0: lo3, 1: hi3}, img, 3)
            _emit_section_dmas(nc, out, img, "cA3", lo3, 0, overshoot=osh)
            _emit_section_dmas(nc, out, img, "cH3", hi3, 0, overshoot=osh)
            _emit_section_dmas(nc, out, img, "cV3", lo3, 134, overshoot=osh)
            _emit_section_dmas(nc, out, img, "cD3", hi3, 134)

        STAGGER = 0
        if STAGGER:
            # software pipeline: L1(k) || L2(k-1) || L3(k-2) so the thin
            # L2/L3 dependency chains hide inside the fat L1 stream
            for k in range(len(seq) + 2):
                if k < len(seq):
                    emit_L1(k)
                if 0 <= k - 1 < len(seq):
                    emit_L2(k - 1)
                if 0 <= k - 2 < len(seq):
                    emit_L3(k - 2)
        else:
            for k in range(len(seq)):
                emit_L1(k)
                emit_L2(k)
                emit_L3(k)

    nc.compile()
    return nc


# ----------------------------------------------------------------- runner
def _get_built():
    global _BUILT
    if _BUILT is None:
        _BUILT = build_bass()
    return _BUILT


def kernel(x: np.ndarray) -> np.ndarray:
    from concourse import bass_utils

    x = np.asarray(x)
    assert x.shape == (B, C, H, W), x.shape
    nc = _get_built()

    imgs = np.ascontiguousarray(x.reshape(B * C, H, W).astype(np.float16))
    in_maps = []
    for c in range(N_CORES):
        m = {"xin": imgs[c * IMGS_PER_CORE:(c + 1) * IMGS_PER_CORE]}
        for N, _, _, _ in LEVELS:
            m[f"w{N}"] = PLANS[N]["warr"]
        in_maps.append(m)

    res = bass_utils.run_bass_kernel_spmd(nc, in_maps,
                                          core_ids=list(range(N_CORES)))
    outs = [np.asarray(res.results[c]["out"], dtype=np.float32)
            for c in range(N_CORES)]
    flat = np.concatenate(outs, axis=0)  # [48, 1048576]
    return flat.reshape(B, C, 64, 128, 128)


# revision 45
# speedup vs baseline: 4.4587x; 1.0302x over previous
"""Trainium2 Bass kernel for nn_DWT_Layer: 3-level 2D db4 DWT (symmetric mode).

Input  x: (16, 3, 1024, 1024) fp32.
Output:   (16, 3, 64, 128, 128) fp32 — the flattened/truncated wavelet pyramid
          [cA3, cH3, cV3, cD3, cH2, cV2, cD2, cH1, cV1, cD1(truncated)].

Sharding: pure data parallel — the 48 (batch*channel) images are split 6 per
NeuronCore across 8 cores; no communication.

All compute runs on the tensor engine in fp16 (1 PE cycle per output row at
any free size). The separable transform per level is two banded matmul
passes with the symmetric extension folded into the weights:

  pass 1 (H):  A^T = X^T · M^T   — lhsT = a 128-col block of X (stationary),
               rhs = a [128, <=64] block of the folded DWT matrix M^T.
               Swapping the stationary operand makes the output land
               TRANSPOSED (image columns on psum partitions), which is
               exactly what pass 2 needs.
  pass 2 (W):  out = A · M^T     — lhsT = a 128-row block of A^T, rhs = the
               SAME weight blocks; output is row-major [h', w'], so the four
               quadrants stream straight to per-section staging tiles and
               then to DRAM in a handful of large DMAs.

Free-dim chunks are 64 wide so each chunk's 8-tap band touches at most two
128-row contraction blocks (2 PE cycles per output element). Intermediates,
weights and output staging are fp16 (validated ~7e-4 rel err vs the fp32
reference); psum accumulation is fp32.
"""
import numpy as np

# ----------------------------------------------------------------- constants
DEC_LO = np.array([-0.010597401784997278, 0.032883011666982945,
                   0.030841381835986965, -0.18703481171888114,
                   -0.027983769416983849, 0.63088076792959036,
                   0.71484657055254153, 0.23037781330885523], dtype=np.float64)
L = 8
DEC_HI = np.array([(-1.0) ** (k + 1) * DEC_LO[L - 1 - k] for k in range(L)],
                  dtype=np.float64)

B, C, H, W = 16, 3, 1024, 1024
N_CORES = 8
IMGS_PER_CORE = 6
IMG_ELEMS = H * W
CH = 32          # free-dim chunk width for the banded matmuls

LEVELS = [  # (N, N', n_slots_in, n_out_tiles) — first field used; rest compat
    (1024, 515, 8, 9),
    (515, 261, 5, 5),
    (261, 134, 3, 3),
]

# output section offsets (elements within one image's 1048576-long output)
SECT = {}
_cur = 0
for _name, _n in [("cA3", 134), ("cH3", 134), ("cV3", 134), ("cD3", 134),
                  ("cH2", 261), ("cV2", 261), ("cD2", 261),
                  ("cH1", 515), ("cV1", 515), ("cD1", 515)]:
    SECT[_name] = (_cur, _n)
    _cur += _n * _n
# cD1 truncation: keep first 469 full rows + 404 elems of row 469
CD1_FULL_ROWS = 469
CD1_PART_COLS = 404
assert SECT["cD1"][0] + CD1_FULL_ROWS * 515 + CD1_PART_COLS == IMG_ELEMS


def nprime(N):
    return (N + 5) // 2 + 1


def ext_index(j, N):
    if j < 6:
        return 5 - j
    if j < N + 6:
        return j - 6
    return 2 * N + 5 - j


def dwt_matrix(N, filt):
    Np = nprime(N)
    M = np.zeros((Np, N), dtype=np.float64)
    filtrev = filt[::-1]
    for i in range(Np):
        for t in range(L):
            M[i, ext_index(2 * i + t, N)] += filtrev[t]
    return M


def level_plan(N):
    """Chunk/weight-block tables for one level (identical for H and W axes).

    chunks: [(sec, out0, w, [(q, bidx)])] over sec in {0:lo, 1:hi}, 64-wide
    output chunks; warr [128, nblk, CH] packs rhs blocks (contraction rows on
    partitions). groups: chunk-aligned psum column groups <= 512 wide over
    the stacked free axis (lo at [0,Np), hi at [Np,2Np))."""
    Np = nprime(N)
    n_cb = (N + 127) // 128
    mats = (dwt_matrix(N, DEC_LO), dwt_matrix(N, DEC_HI))
    chunks = []
    blocks = []
    for sec in (0, 1):
        M = mats[sec]
        for out0 in range(0, Np, CH):
            w = min(CH, Np - out0)
            qs = []
            for q in range(n_cb):
                qn = min(128, N - 128 * q)
                blk = M[out0:out0 + w, 128 * q:128 * q + qn]
                if np.any(blk != 0):
                    qs.append((q, len(blocks)))
                    blocks.append((qn, w, blk.T.copy()))
            chunks.append((sec, out0, w, qs))
    warr = np.zeros((128, len(blocks), CH), dtype=np.float16)
    for b, (qn, w, data) in enumerate(blocks):
        warr[:qn, b, :w] = data.astype(np.float16)
    # psum groups: chunk-aligned, <=512 wide (each fits one psum bank)
    groups = []
    cur0, cur = None, 0
    for (sec, out0, w, qs) in chunks:
        col0 = sec * Np + out0
        if cur0 is None:
            cur0, cur = col0, w
        elif col0 == cur0 + cur and cur + w <= 512:
            cur += w
        else:
            groups.append((cur0, cur))
            cur0, cur = col0, w
    groups.append((cur0, cur))
    return dict(N=N, Np=Np, n_cb=n_cb, chunks=chunks, warr=warr, groups=groups)


PLANS = {N: level_plan(N) for N, _, _, _ in LEVELS}
WC = {N: (PLANS[N]["warr"],) for N, _, _, _ in LEVELS}   # test.py compat
TAPS_ARR = np.zeros((128, 16), dtype=np.float32)          # unused; compat

_BUILT = None


class _CopySched:
    """Greedy least-loaded assignment of psum->sbuf copies across engines."""

    def __init__(self, nc):
        self.nc = nc
        self.busy = {"dve": 0.0, "act": 0.0}
        self.cost = {
            "dve": lambda e: e * 1.0417 + 130.0,
            "act": lambda e: e * 0.8333 + 190.0,
        }

    MODE = "alt"   # "greedy" | "alt"

    def copy(self, out_ap, in_ap, els):
        if self.MODE == "alt":
            eng = "dve" if self.busy["dve"] <= self.busy["act"] else "act"
        else:
            eng = min(self.busy, key=lambda k: self.busy[k] + self.cost[k](els))
        self.busy[eng] += self.cost[eng](els)
        if eng == "dve":
            self.nc.vector.tensor_copy(out=out_ap, in_=in_ap)
        else:
            self.nc.scalar.copy(out=out_ap, in_=in_ap)


def _emit_level(nc, sched, psp, wsb, plan, Xsb, At, quad_dst, img, lvl):
    """One DWT level: pass-1 (H) into At, pass-2 (W) into quadrant tiles.

    Xsb: input tile [128, n_cb, >=N] fp16, rows r = 128*s + p, cols [0,N).
    At:  [128, n_cb, 2*Np] fp16, At[p, cb, h'] = A[h', 128*cb + p].
    quad_dst[sh] = combined tile [128, n_pb, 2*Np], rows 128*pbr + p, cols
    stacked [lo-w | hi-w] matching the psum column layout.
    """
    N, Np, n_cb = plan["N"], plan["Np"], plan["n_cb"]
    chunks, groups = plan["chunks"], plan["groups"]

    _ps_ctr = [0]

    def group_tiles(kind):
        out = []
        for gi, (g0, gw) in enumerate(groups):
            import concourse.mybir as mybir
            width = 512 if gw > 64 else 64
            t = psp.tile([128, width], mybir.dt.float32, tag=f"psg{gi}",
                         bufs=4 if gi == 0 else 2,
                         name=f"ps{kind}{lvl}_{img}_{gi}_{_ps_ctr[0]}")
            _ps_ctr[0] += 1
            out.append((g0, gw, t))
        return out

    def run_chunks(ps_tiles, lhsT_of, mt):
        """Emit the banded matmuls for every chunk into the group tiles."""
        for (g0, gw, ps) in ps_tiles:
            for (sec, out0, w, qs) in chunks:
                col0 = sec * Np + out0
                if not (g0 <= col0 < g0 + gw):
                    continue
                for ki, (q, bidx) in enumerate(qs):
                    qn = min(128, N - 128 * q)
                    nc.tensor.matmul(
                        ps[0:mt, col0 - g0:col0 - g0 + w],
                        lhsT_of(q, qn),
                        wsb[0:qn, bidx, 0:w],
                        start=(ki == 0), stop=(ki == len(qs) - 1))

    # ---------------- pass 1: A^T[c, h'] ----------------
    for cb in range(n_cb):
        cw = min(128, N - 128 * cb)
        ps_tiles = group_tiles("1")
        run_chunks(ps_tiles,
                   lambda q, qn: Xsb[0:qn, q, 128 * cb:128 * cb + cw], cw)
        for (g0, gw, ps) in ps_tiles:
            sched.copy(At[0:cw, cb, g0:g0 + gw], ps[0:cw, 0:gw], gw)

    # ---------------- pass 2: out[h', w'] ----------------
    n_pb = (Np + 127) // 128
    for sh in (0, 1):
        dst = quad_dst[sh]   # combined [128, n_pb, 2*Np]: lo-w | hi-w halves
        for pbr in range(n_pb):
            a = sh * Np + 128 * pbr
            pw = min(128, Np - 128 * pbr)
            ps_tiles = group_tiles("2")
            run_chunks(ps_tiles,
                       lambda q, qn: At[0:qn, q, a:a + pw], pw)
            for (g0, gw, ps) in ps_tiles:
                sched.copy(dst[0:pw, pbr, g0:g0 + gw], ps[0:pw, 0:gw], gw)


def _emit_section_dmas(nc, out, img, name, stg, c0, overshoot=False):
    """DMA one output section from staging cols [c0, c0+Wd) of `stg`.

    overshoot=True rounds the row count up to a slot multiple in ONE DMA;
    the spill rows land in the next DRAM section, whose own (later-emitted)
    DMA overwrites them. Only valid when that section's DMA is emitted
    after this one."""
    base, Wd = SECT[name]
    sl = stg[:, :, c0:c0 + Wd]
    if name == "cD1":
        # rows 0..383 bulk, slot-3 rows 384..468, partial row 469 (404 cols)
        dst = out[img, base:base + 3 * 128 * Wd].rearrange(
            "(s p w) -> p s w", p=128, s=3)
        nc.sync.dma_start(out=dst, in_=sl[:, 0:3, :])
        n85 = CD1_FULL_ROWS - 384
        dst = out[img, base + 384 * Wd:base + CD1_FULL_ROWS * Wd].rearrange(
            "(p w) -> p w", w=Wd)
        nc.sync.dma_start(out=dst, in_=sl[0:n85, 3, :])
        dst = out[img, base + CD1_FULL_ROWS * Wd:
                  base + CD1_FULL_ROWS * Wd + CD1_PART_COLS]
        nc.sync.dma_start(out=dst.rearrange("(p w) -> p w", w=CD1_PART_COLS),
                          in_=sl[n85:n85 + 1, 3, 0:CD1_PART_COLS])
        return
    fs, rem = Wd // 128, Wd % 128
    if rem and overshoot:
        dst = out[img, base:base + (fs + 1) * 128 * Wd].rearrange(
            "(s p w) -> p s w", p=128, s=fs + 1)
        nc.sync.dma_start(out=dst, in_=sl[:, 0:fs + 1, :])
        return
    dst = out[img, base:base + fs * 128 * Wd].rearrange(
        "(s p w) -> p s w", p=128, s=fs)
    nc.sync.dma_start(out=dst, in_=sl[:, 0:fs, :])
    if rem:
        dst = out[img, base + fs * 128 * Wd:base + Wd * Wd].rearrange(
            "(p w) -> p w", w=Wd)
        nc.sync.dma_start(out=dst, in_=sl[0:rem, fs, :])


def build_bass(n_images=IMGS_PER_CORE, repeats=1):
    import concourse.mybir as mybir
    import concourse.tile as tile
    from concourse import bacc
    from contextlib import ExitStack

    nc = bacc.Bacc("TRN2", target_bir_lowering=False, debug=False)
    f16 = mybir.dt.float16

    xin = nc.dram_tensor("xin", (n_images, H, W), f16,
                         kind="ExternalInput").ap()
    out = nc.dram_tensor("out", (n_images, IMG_ELEMS), f16,
                         kind="ExternalOutput").ap()
    wdram = {}
    for N, _, _, _ in LEVELS:
        arr = PLANS[N]["warr"]
        wdram[N] = nc.dram_tensor(f"w{N}", arr.shape, f16,
                                  kind="ExternalInput").ap()

    with tile.TileContext(nc) as tc, ExitStack() as ctx:
        cpool = ctx.enter_context(tc.tile_pool(name="consts", bufs=1))
        xp = ctx.enter_context(tc.tile_pool(name="xp", bufs=1))
        atp = ctx.enter_context(tc.tile_pool(name="atp", bufs=1))
        stp = ctx.enter_context(tc.tile_pool(name="stp", bufs=1))
        psp = ctx.enter_context(tc.tile_pool(name="ps", bufs=1, space="PSUM"))

        x1_tiles = {}

        def get_x1(rep, img):
            if img >= n_images:
                return None
            if (rep, img) not in x1_tiles:
                t = xp.tile([128, 8, 1024], f16, tag="x1", bufs=3,
                            name=f"x1_{rep}_{img}")
                src = xin[img].rearrange("(s p) w -> p s w", p=128)
                # quarters: early pass-1 start + lets output DMAs interleave
                for h in range(4):
                    nc.sync.dma_start(out=t[:, 2 * h:2 * h + 2, :],
                                      in_=src[:, 2 * h:2 * h + 2, :])
                x1_tiles[(rep, img)] = t
            return x1_tiles[(rep, img)]

        wsb = {}

        def load_w(N):
            arr = PLANS[N]["warr"]
            wsb[N] = cpool.tile(list(arr.shape), f16, name=f"wsb{N}")
            # contiguous per-partition transfer (4KB runs, not 128B)
            nc.sync.dma_start(
                out=wsb[N][:].rearrange("p b c -> p (b c)"),
                in_=wdram[N].rearrange("p b c -> p (b c)"))

        load_w(1024)   # needed first
        get_x1(0, 0)   # first image input right behind it
        load_w(515)
        load_w(261)

        seq = [(r, i) for r in range(repeats) for i in range(n_images)]
        sched = _CopySched(nc)
        lo1s, lo2s = {}, {}

        def emit_L1(k):
            rep, img = seq[k]
            x1 = get_x1(rep, img)
            # combined lo/hi pair tiles: lo = [aa | cV], hi = [cH | cD];
            # the aa half (cols [0,Np)) doubles as the next level's input.
            at1 = atp.tile([128, 8, 1030], f16, tag="at1", bufs=2,
                           name=f"at1_{rep}_{img}")
            lo1 = xp.tile([128, 5, 1030], f16, tag="lo1", bufs=3,
                          name=f"lo1_{rep}_{img}")
            hi1 = stp.tile([128, 5, 1030], f16, tag="hi1", bufs=2,
                           name=f"hi1_{rep}_{img}")
            _emit_level(nc, sched, psp, wsb[1024], PLANS[1024], x1, at1,
                        {0: lo1, 1: hi1}, img, 1)
            if k + 1 < len(seq):   # prefetch next image's input
                get_x1(*seq[k + 1])
            osh = k + 1 < len(seq)
            _emit_section_dmas(nc, out, img, "cH1", hi1, 0, overshoot=osh)
            _emit_section_dmas(nc, out, img, "cV1", lo1, 515, overshoot=osh)
            _emit_section_dmas(nc, out, img, "cD1", hi1, 515)
            lo1s[k] = lo1

        def emit_L2(k):
            rep, img = seq[k]
            osh = k + 1 < len(seq)
            at2 = atp.tile([128, 5, 522], f16, tag="at2", bufs=2,
                           name=f"at2_{rep}_{img}")
            lo2 = xp.tile([128, 3, 522], f16, tag="lo2", bufs=3,
                          name=f"lo2_{rep}_{img}")
            hi2 = stp.tile([128, 3, 522], f16, tag="hi2", bufs=2,
                           name=f"hi2_{rep}_{img}")
            _emit_level(nc, sched, psp, wsb[515], PLANS[515], lo1s.pop(k),
                        at2, {0: lo2, 1: hi2}, img, 2)
            _emit_section_dmas(nc, out, img, "cH2", hi2, 0, overshoot=osh)
            _emit_section_dmas(nc, out, img, "cV2", lo2, 261, overshoot=osh)
            _emit_section_dmas(nc, out, img, "cD2", hi2, 261)
            lo2s[k] = lo2

        def emit_L3(k):
            rep, img = seq[k]
            osh = k + 1 < len(seq)
            at3 = atp.tile([128, 3, 268], f16, tag="at3", bufs=2,
                           name=f"at3_{rep}_{img}")
            lo3 = stp.tile([128, 2, 268], f16, tag="lo3", bufs=2,
                           name=f"lo3_{rep}_{img}")
            hi3 = stp.tile([128, 2, 268], f16, tag="hi3", bufs=2,
                           name=f"hi3_{rep}_{img}")
            _emit_level(nc, sched, psp, wsb[261], PLANS[261], lo2s.pop(k),
                        at3, {0: lo3, 1: hi3}, img, 3)
            _emit_section_dmas(nc, out, img, "cA3", lo3, 0, overshoot=osh)
            _emit_section_dmas(nc, out, img, "cH3", hi3, 0, overshoot=osh)
            _emit_section_dmas(nc, out, img, "cV3", lo3, 134, overshoot=osh)
            _emit_section_dmas(nc, out, img, "cD3", hi3, 134)

        STAGGER = 0
        if STAGGER:
            # software pipeline: L1(k) || L2(k-1) || L3(k-2) so the thin
            # L2/L3 dependency chains hide inside the fat L1 stream
            for k in range(len(seq) + 2):
                if k < len(seq):
                    emit_L1(k)
                if 0 <= k - 1 < len(seq):
                    emit_L2(k - 1)
                if 0 <= k - 2 < len(seq):
                    emit_L3(k - 2)
        else:
            for k in range(len(seq)):
                emit_L1(k)
                emit_L2(k)
                emit_L3(k)

    nc.compile()
    return nc


# ----------------------------------------------------------------- runner
def _get_built():
    global _BUILT
    if _BUILT is None:
        _BUILT = build_bass()
    return _BUILT


def kernel(x: np.ndarray) -> np.ndarray:
    from concourse import bass_utils

    x = np.asarray(x)
    assert x.shape == (B, C, H, W), x.shape
    nc = _get_built()

    imgs = np.ascontiguousarray(x.reshape(B * C, H, W).astype(np.float16))
    in_maps = []
    for c in range(N_CORES):
        m = {"xin": imgs[c * IMGS_PER_CORE:(c + 1) * IMGS_PER_CORE]}
        for N, _, _, _ in LEVELS:
            m[f"w{N}"] = PLANS[N]["warr"]
        in_maps.append(m)

    res = bass_utils.run_bass_kernel_spmd(nc, in_maps,
                                          core_ids=list(range(N_CORES)))
    outs = [np.asarray(res.results[c]["out"], dtype=np.float32)
            for c in range(N_CORES)]
    flat = np.concatenate(outs, axis=0)  # [48, 1048576]
    return flat.reshape(B, C, 64, 128, 128)


# revision 50
# speedup vs baseline: 4.4987x; 1.0090x over previous
"""Trainium2 Bass kernel for nn_DWT_Layer: 3-level 2D db4 DWT (symmetric mode).

Input  x: (16, 3, 1024, 1024) fp32.
Output:   (16, 3, 64, 128, 128) fp32 — the flattened/truncated wavelet pyramid
          [cA3, cH3, cV3, cD3, cH2, cV2, cD2, cH1, cV1, cD1(truncated)].

Sharding: pure data parallel — the 48 (batch*channel) images are split 6 per
NeuronCore across 8 cores; no communication.

All compute runs on the tensor engine in fp16 (1 PE cycle per output row at
any free size). The separable transform per level is two banded matmul
passes with the symmetric extension folded into the weights:

  pass 1 (H):  A^T = X^T · M^T   — lhsT = a 128-col block of X (stationary),
               rhs = a [128, <=64] block of the folded DWT matrix M^T.
               Swapping the stationary operand makes the output land
               TRANSPOSED (image columns on psum partitions), which is
               exactly what pass 2 needs.
  pass 2 (W):  out = A · M^T     — lhsT = a 128-row block of A^T, rhs = the
               SAME weight blocks; output is row-major [h', w'], so the four
               quadrants stream straight to per-section staging tiles and
               then to DRAM in a handful of large DMAs.

Free-dim chunks are CH=16 wide so each chunk's 8-tap band touches ~1.25
128-row contraction blocks on average. The psum->sbuf copies (DVE+ACT,
split by a least-loaded scheduler) are the critical path; output sections
ship via a few large DMAs alternated between the SP (HWDGE) and Pool
(SWDGE) queues, with slot-rounded "overshoot" writes whose spill rows are
overwritten by the next section's DMA. Intermediates, weights and output
staging are fp16 (validated ~6e-4 rel err vs the fp32 reference); psum
accumulation is fp32.
"""
import numpy as np

# ----------------------------------------------------------------- constants
DEC_LO = np.array([-0.010597401784997278, 0.032883011666982945,
                   0.030841381835986965, -0.18703481171888114,
                   -0.027983769416983849, 0.63088076792959036,
                   0.71484657055254153, 0.23037781330885523], dtype=np.float64)
L = 8
DEC_HI = np.array([(-1.0) ** (k + 1) * DEC_LO[L - 1 - k] for k in range(L)],
                  dtype=np.float64)

B, C, H, W = 16, 3, 1024, 1024
N_CORES = 8
IMGS_PER_CORE = 6
IMG_ELEMS = H * W
CH = 16          # free-dim chunk width for the banded matmuls

LEVELS = [  # (N, N', n_slots_in, n_out_tiles) — first field used; rest compat
    (1024, 515, 8, 9),
    (515, 261, 5, 5),
    (261, 134, 3, 3),
]

# output section offsets (elements within one image's 1048576-long output)
SECT = {}
_cur = 0
for _name, _n in [("cA3", 134), ("cH3", 134), ("cV3", 134), ("cD3", 134),
                  ("cH2", 261), ("cV2", 261), ("cD2", 261),
                  ("cH1", 515), ("cV1", 515), ("cD1", 515)]:
    SECT[_name] = (_cur, _n)
    _cur += _n * _n
# cD1 truncation: keep first 469 full rows + 404 elems of row 469
CD1_FULL_ROWS = 469
CD1_PART_COLS = 404
assert SECT["cD1"][0] + CD1_FULL_ROWS * 515 + CD1_PART_COLS == IMG_ELEMS


def nprime(N):
    return (N + 5) // 2 + 1


def ext_index(j, N):
    if j < 6:
        return 5 - j
    if j < N + 6:
        return j - 6
    return 2 * N + 5 - j


def dwt_matrix(N, filt):
    Np = nprime(N)
    M = np.zeros((Np, N), dtype=np.float64)
    filtrev = filt[::-1]
    for i in range(Np):
        for t in range(L):
            M[i, ext_index(2 * i + t, N)] += filtrev[t]
    return M


def level_plan(N):
    """Chunk/weight-block tables for one level (identical for H and W axes).

    chunks: [(sec, out0, w, [(q, bidx)])] over sec in {0:lo, 1:hi}, 64-wide
    output chunks; warr [128, nblk, CH] packs rhs blocks (contraction rows on
    partitions). groups: chunk-aligned psum column groups <= 512 wide over
    the stacked free axis (lo at [0,Np), hi at [Np,2Np))."""
    Np = nprime(N)
    n_cb = (N + 127) // 128
    mats = (dwt_matrix(N, DEC_LO), dwt_matrix(N, DEC_HI))
    chunks = []
    blocks = []
    for sec in (0, 1):
        M = mats[sec]
        for out0 in range(0, Np, CH):
            w = min(CH, Np - out0)
            qs = []
            for q in range(n_cb):
                qn = min(128, N - 128 * q)
                blk = M[out0:out0 + w, 128 * q:128 * q + qn]
                if np.any(blk != 0):
                    qs.append((q, len(blocks)))
                    blocks.append((qn, w, blk.T.copy()))
            chunks.append((sec, out0, w, qs))
    warr = np.zeros((128, len(blocks), CH), dtype=np.float16)
    for b, (qn, w, data) in enumerate(blocks):
        warr[:qn, b, :w] = data.astype(np.float16)
    # psum groups: chunk-aligned, <=512 wide (each fits one psum bank)
    groups = []
    cur0, cur = None, 0
    for (sec, out0, w, qs) in chunks:
        col0 = sec * Np + out0
        if cur0 is None:
            cur0, cur = col0, w
        elif col0 == cur0 + cur and cur + w <= 512:
            cur += w
        else:
            groups.append((cur0, cur))
            cur0, cur = col0, w
    groups.append((cur0, cur))
    return dict(N=N, Np=Np, n_cb=n_cb, chunks=chunks, warr=warr, groups=groups)


PLANS = {N: level_plan(N) for N, _, _, _ in LEVELS}
WC = {N: (PLANS[N]["warr"],) for N, _, _, _ in LEVELS}   # test.py compat
TAPS_ARR = np.zeros((128, 16), dtype=np.float32)          # unused; compat

_BUILT = None


class _CopySched:
    """Greedy least-loaded assignment of psum->sbuf copies across engines."""

    def __init__(self, nc):
        self.nc = nc
        self.busy = {"dve": 0.0, "act": 0.0}
        self.cost = {
            "dve": lambda e: e * 1.0417 + 130.0,
            "act": lambda e: e * 0.8333 + 190.0,
        }

    MODE = "greedy"   # "greedy" | "alt"

    def copy(self, out_ap, in_ap, els):
        if self.MODE == "alt":
            eng = "dve" if self.busy["dve"] <= self.busy["act"] else "act"
        else:
            eng = min(self.busy, key=lambda k: self.busy[k] + self.cost[k](els))
        self.busy[eng] += self.cost[eng](els)
        if eng == "dve":
            self.nc.vector.tensor_copy(out=out_ap, in_=in_ap)
        else:
            self.nc.scalar.copy(out=out_ap, in_=in_ap)


def _emit_level(nc, sched, psp, wsb, plan, Xsb, At, quad_dst, img, lvl):
    """One DWT level: pass-1 (H) into At, pass-2 (W) into quadrant tiles.

    Xsb: input tile [128, n_cb, >=N] fp16, rows r = 128*s + p, cols [0,N).
    At:  [128, n_cb, 2*Np] fp16, At[p, cb, h'] = A[h', 128*cb + p].
    quad_dst[sh] = combined tile [128, n_pb, 2*Np], rows 128*pbr + p, cols
    stacked [lo-w | hi-w] matching the psum column layout.
    """
    N, Np, n_cb = plan["N"], plan["Np"], plan["n_cb"]
    chunks, groups = plan["chunks"], plan["groups"]

    _ps_ctr = [0]

    def group_tiles(kind):
        out = []
        for gi, (g0, gw) in enumerate(groups):
            import concourse.mybir as mybir
            width = 512 if gw > 64 else 64
            t = psp.tile([128, width], mybir.dt.float32, tag=f"psg{gi}",
                         bufs=4 if gi == 0 else 2,
                         name=f"ps{kind}{lvl}_{img}_{gi}_{_ps_ctr[0]}")
            _ps_ctr[0] += 1
            out.append((g0, gw, t))
        return out

    def run_chunks(ps_tiles, lhsT_of, mt):
        """Emit the banded matmuls for every chunk into the group tiles."""
        for (g0, gw, ps) in ps_tiles:
            for (sec, out0, w, qs) in chunks:
                col0 = sec * Np + out0
                if not (g0 <= col0 < g0 + gw):
                    continue
                for ki, (q, bidx) in enumerate(qs):
                    qn = min(128, N - 128 * q)
                    nc.tensor.matmul(
                        ps[0:mt, col0 - g0:col0 - g0 + w],
                        lhsT_of(q, qn),
                        wsb[0:qn, bidx, 0:w],
                        start=(ki == 0), stop=(ki == len(qs) - 1))

    # ---------------- pass 1: A^T[c, h'] ----------------
    for cb in range(n_cb):
        cw = min(128, N - 128 * cb)
        ps_tiles = group_tiles("1")
        run_chunks(ps_tiles,
                   lambda q, qn: Xsb[0:qn, q, 128 * cb:128 * cb + cw], cw)
        for (g0, gw, ps) in ps_tiles:
            sched.copy(At[0:cw, cb, g0:g0 + gw], ps[0:cw, 0:gw], gw)

    # ---------------- pass 2: out[h', w'] ----------------
    n_pb = (Np + 127) // 128
    for sh in (0, 1):
        dst = quad_dst[sh]   # combined [128, n_pb, 2*Np]: lo-w | hi-w halves
        for pbr in range(n_pb):
            a = sh * Np + 128 * pbr
            pw = min(128, Np - 128 * pbr)
            ps_tiles = group_tiles("2")
            run_chunks(ps_tiles,
                       lambda q, qn: At[0:qn, q, a:a + pw], pw)
            for (g0, gw, ps) in ps_tiles:
                sched.copy(dst[0:pw, pbr, g0:g0 + gw], ps[0:pw, 0:gw], gw)


def _emit_section_dmas(nc, out, img, name, stg, c0, overshoot=False):
    """DMA one output section from staging cols [c0, c0+Wd) of `stg`.

    overshoot=True rounds the row count up to a slot multiple in ONE DMA;
    the spill rows land in the next DRAM section, whose own (later-emitted)
    DMA overwrites them. Only valid when that section's DMA is emitted
    after this one."""
    base, Wd = SECT[name]
    sl = stg[:, :, c0:c0 + Wd]
    if name == "cD1":
        # rows 0..383 bulk, slot-3 rows 384..468, partial row 469 (404 cols)
        dst = out[img, base:base + 3 * 128 * Wd].rearrange(
            "(s p w) -> p s w", p=128, s=3)
        nc.sync.dma_start(out=dst, in_=sl[:, 0:3, :])
        n85 = CD1_FULL_ROWS - 384
        dst = out[img, base + 384 * Wd:base + CD1_FULL_ROWS * Wd].rearrange(
            "(p w) -> p w", w=Wd)
        nc.sync.dma_start(out=dst, in_=sl[0:n85, 3, :])
        dst = out[img, base + CD1_FULL_ROWS * Wd:
                  base + CD1_FULL_ROWS * Wd + CD1_PART_COLS]
        nc.sync.dma_start(out=dst.rearrange("(p w) -> p w", w=CD1_PART_COLS),
                          in_=sl[n85:n85 + 1, 3, 0:CD1_PART_COLS])
        return
    fs, rem = Wd // 128, Wd % 128
    if rem and overshoot:
        dst = out[img, base:base + (fs + 1) * 128 * Wd].rearrange(
            "(s p w) -> p s w", p=128, s=fs + 1)
        nc.sync.dma_start(out=dst, in_=sl[:, 0:fs + 1, :])
        return
    dst = out[img, base:base + fs * 128 * Wd].rearrange(
        "(s p w) -> p s w", p=128, s=fs)
    nc.sync.dma_start(out=dst, in_=sl[:, 0:fs, :])
    if rem:
        dst = out[img, base + fs * 128 * Wd:base + Wd * Wd].rearrange(
            "(p w) -> p w", w=Wd)
        nc.sync.dma_start(out=dst, in_=sl[0:rem, fs, :])


def build_bass(n_images=IMGS_PER_CORE, repeats=1):
    import concourse.mybir as mybir
    import concourse.tile as tile
    from concourse import bacc
    from contextlib import ExitStack

    nc = bacc.Bacc("TRN2", target_bir_lowering=False, debug=False)
    f16 = mybir.dt.float16

    xin = nc.dram_tensor("xin", (n_images, H, W), f16,
                         kind="ExternalInput").ap()
    out = nc.dram_tensor("out", (n_images, IMG_ELEMS), f16,
                         kind="ExternalOutput").ap()
    wdram = {}
    for N, _, _, _ in LEVELS:
        arr = PLANS[N]["warr"]
        wdram[N] = nc.dram_tensor(f"w{N}", arr.shape, f16,
                                  kind="ExternalInput").ap()

    with tile.TileContext(nc) as tc, ExitStack() as ctx:
        cpool = ctx.enter_context(tc.tile_pool(name="consts", bufs=1))
        xp = ctx.enter_context(tc.tile_pool(name="xp", bufs=1))
        atp = ctx.enter_context(tc.tile_pool(name="atp", bufs=1))
        stp = ctx.enter_context(tc.tile_pool(name="stp", bufs=1))
        psp = ctx.enter_context(tc.tile_pool(name="ps", bufs=1, space="PSUM"))

        x1_tiles = {}

        def get_x1(rep, img):
            if img >= n_images:
                return None
            if (rep, img) not in x1_tiles:
                t = xp.tile([128, 8, 1024], f16, tag="x1", bufs=3,
                            name=f"x1_{rep}_{img}")
                src = xin[img].rearrange("(s p) w -> p s w", p=128)
                # quarters: early pass-1 start + lets output DMAs interleave
                for h in range(8):
                    nc.sync.dma_start(out=t[:, h:h + 1, :],
                                      in_=src[:, h:h + 1, :])
                x1_tiles[(rep, img)] = t
            return x1_tiles[(rep, img)]

        wsb = {}

        def load_w(N):
            arr = PLANS[N]["warr"]
            wsb[N] = cpool.tile(list(arr.shape), f16, name=f"wsb{N}")
            # contiguous per-partition transfer (4KB runs, not 128B)
            nc.sync.dma_start(
                out=wsb[N][:].rearrange("p b c -> p (b c)"),
                in_=wdram[N].rearrange("p b c -> p (b c)"))

        load_w(1024)   # needed first
        get_x1(0, 0)   # first image input right behind it
        load_w(515)
        load_w(261)

        seq = [(r, i) for r in range(repeats) for i in range(n_images)]
        sched = _CopySched(nc)
        lo1s, lo2s = {}, {}

        def emit_L1(k):
            rep, img = seq[k]
            x1 = get_x1(rep, img)
            # combined lo/hi pair tiles: lo = [aa | cV], hi = [cH | cD];
            # the aa half (cols [0,Np)) doubles as the next level's input.
            at1 = atp.tile([128, 8, 1030], f16, tag="at1", bufs=2,
                           name=f"at1_{rep}_{img}")
            lo1 = xp.tile([128, 5, 1030], f16, tag="lo1", bufs=3,
                          name=f"lo1_{rep}_{img}")
            hi1 = stp.tile([128, 5, 1030], f16, tag="hi1", bufs=2,
                           name=f"hi1_{rep}_{img}")
            _emit_level(nc, sched, psp, wsb[1024], PLANS[1024], x1, at1,
                        {0: lo1, 1: hi1}, img, 1)
            if k + 1 < len(seq):   # prefetch next image's input
                get_x1(*seq[k + 1])
            osh = k + 1 < len(seq)
            _emit_section_dmas(nc, out, img, "cH1", hi1, 0, overshoot=osh)
            _emit_section_dmas(nc, out, img, "cV1", lo1, 515, overshoot=osh)
            _emit_section_dmas(nc, out, img, "cD1", hi1, 515)
            lo1s[k] = lo1

        def emit_L2(k):
            rep, img = seq[k]
            osh = k + 1 < len(seq)
            at2 = atp.tile([128, 5, 522], f16, tag="at2", bufs=2,
                           name=f"at2_{rep}_{img}")
            lo2 = xp.tile([128, 3, 522], f16, tag="lo2", bufs=3,
                          name=f"lo2_{rep}_{img}")
            hi2 = stp.tile([128, 3, 522], f16, tag="hi2", bufs=2,
                           name=f"hi2_{rep}_{img}")
            _emit_level(nc, sched, psp, wsb[515], PLANS[515], lo1s.pop(k),
                        at2, {0: lo2, 1: hi2}, img, 2)
            _emit_section_dmas(nc, out, img, "cH2", hi2, 0, overshoot=osh)
            _emit_section_dmas(nc, out, img, "cV2", lo2, 261, overshoot=osh)
            _emit_section_dmas(nc, out, img, "cD2", hi2, 261)
            lo2s[k] = lo2

        def emit_L3(k):
            rep, img = seq[k]
            osh = k + 1 < len(seq)
            at3 = atp.tile([128, 3, 268], f16, tag="at3", bufs=2,
                           name=f"at3_{rep}_{img}")
            lo3 = stp.tile([128, 2, 268], f16, tag="lo3", bufs=2,
                           name=f"lo3_{rep}_{img}")
            hi3 = stp.tile([128, 2, 268], f16, tag="hi3", bufs=2,
                           name=f"hi3_{rep}_{img}")
            _emit_level(nc, sched, psp, wsb[261], PLANS[261], lo2s.pop(k),
                        at3, {0: lo3, 1: hi3}, img, 3)
            _emit_section_dmas(nc, out, img, "cA3", lo3, 0, overshoot=osh)
            _emit_section_dmas(nc, out, img, "cH3", hi3, 0, overshoot=osh)
            _emit_section_dmas(nc, out, img, "cV3", lo3, 134, overshoot=osh)
            _emit_section_dmas(nc, out, img, "cD3", hi3, 134)

        STAGGER = 0
        if STAGGER:
            # software pipeline: L1(k) || L2(k-1) || L3(k-2) so the thin
            # L2/L3 dependency chains hide inside the fat L1 stream
            for k in range(len(seq) + 2):
                if k < len(seq):
                    emit_L1(k)
                if 0 <= k - 1 < len(seq):
                    emit_L2(k - 1)
                if 0 <= k - 2 < len(seq):
                    emit_L3(k - 2)
        else:
            for k in range(len(seq)):
                emit_L1(k)
                emit_L2(k)
                emit_L3(k)

    nc.compile()
    return nc


# ----------------------------------------------------------------- runner
def _get_built():
    global _BUILT
    if _BUILT is None:
        _BUILT = build_bass()
    return _BUILT


def kernel(x: np.ndarray) -> np.ndarray:
    from concourse import bass_utils

    x = np.asarray(x)
    assert x.shape == (B, C, H, W), x.shape
    nc = _get_built()

    imgs = np.ascontiguousarray(x.reshape(B * C, H, W).astype(np.float16))
    in_maps = []
    for c in range(N_CORES):
        m = {"xin": imgs[c * IMGS_PER_CORE:(c + 1) * IMGS_PER_CORE]}
        for N, _, _, _ in LEVELS:
            m[f"w{N}"] = PLANS[N]["warr"]
        in_maps.append(m)

    res = bass_utils.run_bass_kernel_spmd(nc, in_maps,
                                          core_ids=list(range(N_CORES)))
    outs = [np.asarray(res.results[c]["out"], dtype=np.float32)
            for c in range(N_CORES)]
    flat = np.concatenate(outs, axis=0)  # [48, 1048576]
    return flat.reshape(B, C, 64, 128, 128)


# revision 67
# speedup vs baseline: 4.5297x; 1.0069x over previous
"""Trainium2 Bass kernel for nn_DWT_Layer: 3-level 2D db4 DWT (symmetric mode).

Input  x: (16, 3, 1024, 1024) fp32.
Output:   (16, 3, 64, 128, 128) fp32 — the flattened/truncated wavelet pyramid
          [cA3, cH3, cV3, cD3, cH2, cV2, cD2, cH1, cV1, cD1(truncated)].

Sharding: pure data parallel — the 48 (batch*channel) images are split 6 per
NeuronCore across 8 cores; no communication.

All compute runs on the tensor engine in fp16 (1 PE cycle per output row at
any free size). The separable transform per level is two banded matmul
passes with the symmetric extension folded into the weights:

  pass 1 (H):  A^T = X^T · M^T   — lhsT = a 128-col block of X (stationary),
               rhs = a [128, <=64] block of the folded DWT matrix M^T.
               Swapping the stationary operand makes the output land
               TRANSPOSED (image columns on psum partitions), which is
               exactly what pass 2 needs.
  pass 2 (W):  out = A · M^T     — lhsT = a 128-row block of A^T, rhs = the
               SAME weight blocks; output is row-major [h', w'], so the four
               quadrants stream straight to per-section staging tiles and
               then to DRAM in a handful of large DMAs.

Free-dim chunks are 64 wide so each chunk's 8-tap band touches at most two
128-row contraction blocks (2 PE cycles per output element). Intermediates,
weights and output staging are fp16 (validated ~7e-4 rel err vs the fp32
reference); psum accumulation is fp32.
"""
import numpy as np

# ----------------------------------------------------------------- constants
DEC_LO = np.array([-0.010597401784997278, 0.032883011666982945,
                   0.030841381835986965, -0.18703481171888114,
                   -0.027983769416983849, 0.63088076792959036,
                   0.71484657055254153, 0.23037781330885523], dtype=np.float64)
L = 8
DEC_HI = np.array([(-1.0) ** (k + 1) * DEC_LO[L - 1 - k] for k in range(L)],
                  dtype=np.float64)

B, C, H, W = 16, 3, 1024, 1024
N_CORES = 8
IMGS_PER_CORE = 6
IMG_ELEMS = H * W
CH = 16          # free-dim chunk width for the banded matmuls

LEVELS = [  # (N, N', n_slots_in, n_out_tiles) — first field used; rest compat
    (1024, 515, 8, 9),
    (515, 261, 5, 5),
    (261, 134, 3, 3),
]

# output section offsets (elements within one image's 1048576-long output)
SECT = {}
_cur = 0
for _name, _n in [("cA3", 134), ("cH3", 134), ("cV3", 134), ("cD3", 134),
                  ("cH2", 261), ("cV2", 261), ("cD2", 261),
                  ("cH1", 515), ("cV1", 515), ("cD1", 515)]:
    SECT[_name] = (_cur, _n)
    _cur += _n * _n
# cD1 truncation: keep first 469 full rows + 404 elems of row 469
CD1_FULL_ROWS = 469
CD1_PART_COLS = 404
assert SECT["cD1"][0] + CD1_FULL_ROWS * 515 + CD1_PART_COLS == IMG_ELEMS


def nprime(N):
    return (N + 5) // 2 + 1


def ext_index(j, N):
    if j < 6:
        return 5 - j
    if j < N + 6:
        return j - 6
    return 2 * N + 5 - j


def dwt_matrix(N, filt):
    Np = nprime(N)
    M = np.zeros((Np, N), dtype=np.float64)
    filtrev = filt[::-1]
    for i in range(Np):
        for t in range(L):
            M[i, ext_index(2 * i + t, N)] += filtrev[t]
    return M


def plan_from_mats(mats, N):
    """Chunk/weight-block tables for one separable pass pair (H and W axes).

    mats = (lo_mat, hi_mat), each [Np, N] (N = contraction length; for the
    composite direct-level plans N=1024 with multi-level folded matrices).
    chunks: [(sec, out0, w, [(q, bidx)])] over sec in {0:lo, 1:hi}, CH-wide
    output chunks; warr [128, nblk, CH] packs rhs blocks (contraction rows on
    partitions). groups: chunk-aligned psum column groups <= 512 wide over
    the stacked free axis (lo at [0,Np), hi at [Np,2Np))."""
    Np = mats[0].shape[0]
    n_cb = (N + 127) // 128
    chunks = []
    blocks = []
    for sec in (0, 1):
        M = mats[sec]
        for out0 in range(0, Np, CH):
            w = min(CH, Np - out0)
            qs = []
            for q in range(n_cb):
                qn = min(128, N - 128 * q)
                blk = M[out0:out0 + w, 128 * q:128 * q + qn]
                if np.any(blk != 0):
                    qs.append((q, len(blocks)))
                    blocks.append((qn, w, blk.T.copy()))
            chunks.append((sec, out0, w, qs))
    warr = np.zeros((128, len(blocks), CH), dtype=np.float16)
    for b, (qn, w, data) in enumerate(blocks):
        warr[:qn, b, :w] = data.astype(np.float16)
    # psum groups: chunk-aligned, <=512 wide (each fits one psum bank)
    groups = []
    cur0, cur = None, 0
    for (sec, out0, w, qs) in chunks:
        col0 = sec * Np + out0
        if cur0 is None:
            cur0, cur = col0, w
        elif col0 == cur0 + cur and cur + w <= 512:
            cur += w
        else:
            groups.append((cur0, cur))
            cur0, cur = col0, w
    groups.append((cur0, cur))
    return dict(N=N, Np=Np, n_cb=n_cb, chunks=chunks, warr=warr, groups=groups)


def level_plan(N):
    return plan_from_mats((dwt_matrix(N, DEC_LO), dwt_matrix(N, DEC_HI)), N)


PLANS = {N: level_plan(N) for N, _, _, _ in LEVELS}

# composite plans: the LAST image computes L2/L3 directly from the input so
# its three levels are independent streams (collapses the drain-tail chain)
_C2LO = dwt_matrix(515, DEC_LO) @ dwt_matrix(1024, DEC_LO)
_C2HI = dwt_matrix(515, DEC_HI) @ dwt_matrix(1024, DEC_LO)
_C3LO = dwt_matrix(261, DEC_LO) @ _C2LO
_C3HI = dwt_matrix(261, DEC_HI) @ _C2LO
PLAN_L2D = plan_from_mats((_C2LO, _C2HI), 1024)
PLAN_L3D = plan_from_mats((_C3LO, _C3HI), 1024)
WC = {N: (PLANS[N]["warr"],) for N, _, _, _ in LEVELS}   # test.py compat
TAPS_ARR = np.zeros((128, 16), dtype=np.float32)          # unused; compat

_BUILT = None


class _CopySched:
    """Greedy least-loaded assignment of psum->sbuf copies across engines."""

    def __init__(self, nc):
        self.nc = nc
        self.busy = {"dve": 0.0, "act": 0.0}
        self.cost = {
            "dve": lambda e: e * 1.0417 + 130.0,
            "act": lambda e: e * 0.8333 + 190.0,
        }

    MODE = "greedy"   # "greedy" | "alt"

    def copy(self, out_ap, in_ap, els):
        if self.MODE == "alt":
            eng = "dve" if self.busy["dve"] <= self.busy["act"] else "act"
        else:
            eng = min(self.busy, key=lambda k: self.busy[k] + self.cost[k](els))
        self.busy[eng] += self.cost[eng](els)
        if eng == "dve":
            self.nc.vector.tensor_copy(out=out_ap, in_=in_ap)
        else:
            self.nc.scalar.copy(out=out_ap, in_=in_ap)


def _emit_level(nc, sched, psp, wsb, plan, Xsb, At, quad_dst, img, lvl,
                skip_aa=False):
    """One DWT level: pass-1 (H) into At, pass-2 (W) into quadrant tiles.

    Xsb: input tile [128, n_cb, >=N] fp16, rows r = 128*s + p, cols [0,N).
    At:  [128, n_cb, 2*Np] fp16, At[p, cb, h'] = A[h', 128*cb + p].
    quad_dst[sh] = combined tile [128, n_pb, 2*Np], rows 128*pbr + p, cols
    stacked [lo-w | hi-w] matching the psum column layout.
    """
    N, Np, n_cb = plan["N"], plan["Np"], plan["n_cb"]
    chunks, groups = plan["chunks"], plan["groups"]

    _ps_ctr = [0]

    def group_tiles(kind):
        out = []
        for gi, (g0, gw) in enumerate(groups):
            import concourse.mybir as mybir
            width = 512 if gw > 64 else 64
            t = psp.tile([128, width], mybir.dt.float32, tag=f"psg{gi}",
                         bufs=4 if gi == 0 else 2,
                         name=f"ps{kind}{lvl}_{img}_{gi}_{_ps_ctr[0]}")
            _ps_ctr[0] += 1
            out.append((g0, gw, t))
        return out

    def run_chunks(ps_tiles, lhsT_of, mt):
        """Emit the banded matmuls for every chunk into the group tiles."""
        for (g0, gw, ps) in ps_tiles:
            for (sec, out0, w, qs) in chunks:
                col0 = sec * Np + out0
                if not (g0 <= col0 < g0 + gw):
                    continue
                for ki, (q, bidx) in enumerate(qs):
                    qn = min(128, N - 128 * q)
                    nc.tensor.matmul(
                        ps[0:mt, col0 - g0:col0 - g0 + w],
                        lhsT_of(q, qn),
                        wsb[0:qn, bidx, 0:w],
                        start=(ki == 0), stop=(ki == len(qs) - 1))

    # ---------------- pass 1: A^T[c, h'] ----------------
    for cb in range(n_cb):
        cw = min(128, N - 128 * cb)
        ps_tiles = group_tiles("1")
        run_chunks(ps_tiles,
                   lambda q, qn: Xsb[0:qn, q, 128 * cb:128 * cb + cw], cw)
        for (g0, gw, ps) in ps_tiles:
            sched.copy(At[0:cw, cb, g0:g0 + gw], ps[0:cw, 0:gw], gw)

    # ---------------- pass 2: out[h', w'] ----------------
    n_pb = (Np + 127) // 128
    for sh in (0, 1):
        dst = quad_dst[sh]   # combined [128, n_pb, 2*Np]: lo-w | hi-w halves
        for pbr in range(n_pb):
            a = sh * Np + 128 * pbr
            pw = min(128, Np - 128 * pbr)
            ps_tiles = group_tiles("2")
            run_chunks(ps_tiles,
                       lambda q, qn: At[0:qn, q, a:a + pw], pw)
            lo_min = Np if (skip_aa and sh == 0) else 0
            for (g0, gw, ps) in ps_tiles:
                o0 = max(g0, lo_min)
                if o0 >= g0 + gw:
                    continue
                sched.copy(dst[0:pw, pbr, o0:g0 + gw],
                           ps[0:pw, o0 - g0:gw], g0 + gw - o0)


def _emit_section_dmas(nc, out, img, name, stg, c0, overshoot=False):
    """DMA one output section from staging cols [c0, c0+Wd) of `stg`.

    overshoot=True rounds the row count up to a slot multiple in ONE DMA;
    the spill rows land in the next DRAM section, whose own (later-emitted)
    DMA overwrites them. Only valid when that section's DMA is emitted
    after this one."""
    base, Wd = SECT[name]
    sl = stg[:, :, c0:c0 + Wd]
    if name == "cD1":
        # rows 0..383 bulk, slot-3 rows 384..468, partial row 469 (404 cols)
        dst = out[img, base:base + 3 * 128 * Wd].rearrange(
            "(s p w) -> p s w", p=128, s=3)
        nc.sync.dma_start(out=dst, in_=sl[:, 0:3, :])
        n85 = CD1_FULL_ROWS - 384
        dst = out[img, base + 384 * Wd:base + CD1_FULL_ROWS * Wd].rearrange(
            "(p w) -> p w", w=Wd)
        nc.sync.dma_start(out=dst, in_=sl[0:n85, 3, :])
        dst = out[img, base + CD1_FULL_ROWS * Wd:
                  base + CD1_FULL_ROWS * Wd + CD1_PART_COLS]
        nc.sync.dma_start(out=dst.rearrange("(p w) -> p w", w=CD1_PART_COLS),
                          in_=sl[n85:n85 + 1, 3, 0:CD1_PART_COLS])
        return
    fs, rem = Wd // 128, Wd % 128
    if rem and overshoot:
        dst = out[img, base:base + (fs + 1) * 128 * Wd].rearrange(
            "(s p w) -> p s w", p=128, s=fs + 1)
        nc.sync.dma_start(out=dst, in_=sl[:, 0:fs + 1, :])
        return
    dst = out[img, base:base + fs * 128 * Wd].rearrange(
        "(s p w) -> p s w", p=128, s=fs)
    nc.sync.dma_start(out=dst, in_=sl[:, 0:fs, :])
    if rem:
        dst = out[img, base + fs * 128 * Wd:base + Wd * Wd].rearrange(
            "(p w) -> p w", w=Wd)
        nc.sync.dma_start(out=dst, in_=sl[0:rem, fs, :])


def build_bass(n_images=IMGS_PER_CORE, repeats=1):
    import concourse.mybir as mybir
    import concourse.tile as tile
    from concourse import bacc
    from contextlib import ExitStack

    nc = bacc.Bacc("TRN2", target_bir_lowering=False, debug=False)
    f16 = mybir.dt.float16

    xin = nc.dram_tensor("xin", (n_images, H, W), f16,
                         kind="ExternalInput").ap()
    out = nc.dram_tensor("out", (n_images, IMG_ELEMS), f16,
                         kind="ExternalOutput").ap()
    wdram = {}
    for N, _, _, _ in LEVELS:
        arr = PLANS[N]["warr"]
        wdram[N] = nc.dram_tensor(f"w{N}", arr.shape, f16,
                                  kind="ExternalInput").ap()
    wdram["w3d"] = nc.dram_tensor("w3d", PLAN_L3D["warr"].shape, f16,
                                  kind="ExternalInput").ap()

    with tile.TileContext(nc) as tc, ExitStack() as ctx:
        cpool = ctx.enter_context(tc.tile_pool(name="consts", bufs=1))
        xp = ctx.enter_context(tc.tile_pool(name="xp", bufs=1))
        atp = ctx.enter_context(tc.tile_pool(name="atp", bufs=1))
        stp = ctx.enter_context(tc.tile_pool(name="stp", bufs=1))
        psp = ctx.enter_context(tc.tile_pool(name="ps", bufs=1, space="PSUM"))

        x1_tiles = {}

        def get_x1(rep, img):
            if img >= n_images:
                return None
            if (rep, img) not in x1_tiles:
                t = xp.tile([128, 8, 1024], f16, tag="x1", bufs=3,
                            name=f"x1_{rep}_{img}")
                src = xin[img].rearrange("(s p) w -> p s w", p=128)
                # quarters: early pass-1 start + lets output DMAs interleave
                for h in range(8):
                    nc.sync.dma_start(out=t[:, h:h + 1, :],
                                      in_=src[:, h:h + 1, :])
                x1_tiles[(rep, img)] = t
            return x1_tiles[(rep, img)]

        wsb = {}

        def load_w(N):
            arr = PLANS[N]["warr"]
            wsb[N] = cpool.tile(list(arr.shape), f16, name=f"wsb{N}")
            # contiguous per-partition transfer (4KB runs, not 128B)
            nc.sync.dma_start(
                out=wsb[N][:].rearrange("p b c -> p (b c)"),
                in_=wdram[N].rearrange("p b c -> p (b c)"))

        load_w(1024)   # needed first
        get_x1(0, 0)   # first image input right behind it
        load_w(515)
        load_w(261)
        arr = PLAN_L3D["warr"]   # composite weights for the last image
        wsb["w3d"] = cpool.tile(list(arr.shape), f16, name="wsbw3d")
        nc.sync.dma_start(
            out=wsb["w3d"][:].rearrange("p b c -> p (b c)"),
            in_=wdram["w3d"].rearrange("p b c -> p (b c)"))

        seq = [(r, i) for r in range(repeats) for i in range(n_images)]
        sched = _CopySched(nc)
        lo1s, lo2s = {}, {}

        def emit_L1(k):
            rep, img = seq[k]
            x1 = get_x1(rep, img)
            # combined lo/hi pair tiles: lo = [aa | cV], hi = [cH | cD];
            # the aa half (cols [0,Np)) doubles as the next level's input.
            at1 = atp.tile([128, 8, 1030], f16, tag="at1", bufs=2,
                           name=f"at1_{rep}_{img}")
            lo1 = xp.tile([128, 5, 1030], f16, tag="lo1", bufs=3,
                          name=f"lo1_{rep}_{img}")
            hi1 = stp.tile([128, 5, 1030], f16, tag="hi1", bufs=2,
                           name=f"hi1_{rep}_{img}")
            _emit_level(nc, sched, psp, wsb[1024], PLANS[1024], x1, at1,
                        {0: lo1, 1: hi1}, img, 1)
            if k + 1 < len(seq):   # prefetch next image's input
                get_x1(*seq[k + 1])
            osh = k + 1 < len(seq)
            _emit_section_dmas(nc, out, img, "cH1", hi1, 0, overshoot=osh)
            _emit_section_dmas(nc, out, img, "cV1", lo1, 515, overshoot=osh)
            _emit_section_dmas(nc, out, img, "cD1", hi1, 515)
            lo1s[k] = lo1

        def emit_L2(k):
            rep, img = seq[k]
            osh = k + 1 < len(seq)
            at2 = atp.tile([128, 5, 522], f16, tag="at2", bufs=2,
                           name=f"at2_{rep}_{img}")
            lo2 = xp.tile([128, 3, 522], f16, tag="lo2", bufs=3,
                          name=f"lo2_{rep}_{img}")
            hi2 = stp.tile([128, 3, 522], f16, tag="hi2", bufs=2,
                           name=f"hi2_{rep}_{img}")
            _emit_level(nc, sched, psp, wsb[515], PLANS[515], lo1s.pop(k),
                        at2, {0: lo2, 1: hi2}, img, 2,
                        skip_aa=(k == len(seq) - 1))
            _emit_section_dmas(nc, out, img, "cH2", hi2, 0, overshoot=osh)
            _emit_section_dmas(nc, out, img, "cV2", lo2, 261, overshoot=osh)
            _emit_section_dmas(nc, out, img, "cD2", hi2, 261)
            lo2s[k] = lo2

        def emit_L3(k):
            rep, img = seq[k]
            osh = k + 1 < len(seq)
            at3 = atp.tile([128, 3, 268], f16, tag="at3", bufs=2,
                           name=f"at3_{rep}_{img}")
            lo3 = stp.tile([128, 2, 268], f16, tag="lo3", bufs=2,
                           name=f"lo3_{rep}_{img}")
            hi3 = stp.tile([128, 2, 268], f16, tag="hi3", bufs=2,
                           name=f"hi3_{rep}_{img}")
            _emit_level(nc, sched, psp, wsb[261], PLANS[261], lo2s.pop(k),
                        at3, {0: lo3, 1: hi3}, img, 3)
            _emit_section_dmas(nc, out, img, "cA3", lo3, 0, overshoot=osh)
            _emit_section_dmas(nc, out, img, "cH3", hi3, 0, overshoot=osh)
            _emit_section_dmas(nc, out, img, "cV3", lo3, 134, overshoot=osh)
            _emit_section_dmas(nc, out, img, "cD3", hi3, 134)

        def emit_L3_direct(k):
            # last image: L3 straight from the raw input via composite
            # matrices, independent of L1/L2 — shortens the drain chain
            rep, img = seq[k]
            x1 = get_x1(rep, img)
            at3d = atp.tile([128, 8, 268], f16, tag="at3d", bufs=1,
                            name=f"at3d_{rep}_{img}")
            lo3 = stp.tile([128, 2, 268], f16, tag="lo3", bufs=2,
                           name=f"lo3d_{rep}_{img}")
            hi3 = stp.tile([128, 2, 268], f16, tag="hi3", bufs=2,
                           name=f"hi3d_{rep}_{img}")
            _emit_level(nc, sched, psp, wsb["w3d"], PLAN_L3D, x1, at3d,
                        {0: lo3, 1: hi3}, img, 3)
            _emit_section_dmas(nc, out, img, "cA3", lo3, 0)
            _emit_section_dmas(nc, out, img, "cH3", hi3, 0)
            _emit_section_dmas(nc, out, img, "cV3", lo3, 134)
            _emit_section_dmas(nc, out, img, "cD3", hi3, 134)

        for k in range(len(seq)):
            if k == len(seq) - 1:
                emit_L3_direct(k)
                emit_L1(k)
                emit_L2(k)
            else:
                emit_L1(k)
                emit_L2(k)
                emit_L3(k)

    nc.compile()
    return nc


# ----------------------------------------------------------------- runner
def _get_built():
    global _BUILT
    if _BUILT is None:
        _BUILT = build_bass()
    return _BUILT


def kernel(x: np.ndarray) -> np.ndarray:
    from concourse import bass_utils

    x = np.asarray(x)
    assert x.shape == (B, C, H, W), x.shape
    nc = _get_built()

    imgs = np.ascontiguousarray(x.reshape(B * C, H, W).astype(np.float16))
    in_maps = []
    for c in range(N_CORES):
        m = {"xin": imgs[c * IMGS_PER_CORE:(c + 1) * IMGS_PER_CORE]}
        for N, _, _, _ in LEVELS:
            m[f"w{N}"] = PLANS[N]["warr"]
        m["w3d"] = PLAN_L3D["warr"]
        in_maps.append(m)

    res = bass_utils.run_bass_kernel_spmd(nc, in_maps,
                                          core_ids=list(range(N_CORES)))
    outs = [np.asarray(res.results[c]["out"], dtype=np.float32)
            for c in range(N_CORES)]
    flat = np.concatenate(outs, axis=0)  # [48, 1048576]
    return flat.reshape(B, C, 64, 128, 128)
